# revision 24
# baseline (speedup 1.0000x reference)
"""BlockAttention Trainium2 kernel.

Full inputs: q (32,16,4,64) f32, k/v (32,8192,4,64) f32.
Outputs: (out (32,16,4,64) f32, tokens (32,16,4) int32).

Sharding: data-parallel over batch, 4 batches per core x 8 cores.

Per-core device algorithm, per batch-group b (the 4 t's of one batch are
processed together, packed on PSUM partition groups {0,32,64,96}):
  - QK^T in bf16 on TensorE (Q stationary [64d,16kq], K^T streamed in 512-v
    chunks), logits*1/8 -> exp via ScalarE (fp32 in PSUM -> bf16 slab in SBUF,
    fused per-partition row-sum accumulation = softmax denominator).
  - PE-transposes of 128-v exp chunks -> attn^T tiles -> bf16 AV matmuls
    accumulated over the whole vocab in PSUM; normalized by reciprocal sum.
  - tokens: top-8 candidates per row via max8/max_index on the exp slab
    (exp is monotonic in the logits), then an *exact fp32 rescore*: the 8
    candidate k-vectors per row are gathered from the untouched fp32 k via
    indirect DMA and re-dotted with q on VectorE; argmax with min-index
    tie-break (matches jnp.argmax first-occurrence).
"""

import sys

import numpy as np

try:
    import concourse.bass as bass  # noqa: F401
except ImportError:  # pragma: no cover
    sys.path.insert(0, "/opt/trn_rl_repo")

import ml_dtypes

import concourse.bass as bass
import concourse.tile as tile
from concourse import bacc, mybir
from concourse.bass_utils import run_bass_kernel_spmd

F32 = mybir.dt.float32
BF16 = mybir.dt.bfloat16
U32 = mybir.dt.uint32

BS, KQ, T, D, V = 32, 16, 4, 64, 8192
NCORES = 8
BPC = BS // NCORES          # batches per core = 4
NCHUNK = 16                 # 512-v chunks per (b,t)
CW = 512                    # chunk width (PSUM bank in fp32)
NSUB = V // 128             # 64 128-v subchunks per (b,t)

_CACHED = {}
FEATURES = {"av", "argmax", "rescore", "gather"}
DEBUG = False


def _build():
    nc = bacc.Bacc("TRN2", num_devices=NCORES)

    ktb = nc.dram_tensor("ktb", [BPC, T, 128, V // 2], BF16, kind="ExternalInput")
    vb = nc.dram_tensor("vb", [BPC, 4, 128, 4096], BF16, kind="ExternalInput")
    qtb = nc.dram_tensor("qtb", [BPC, 128, T * KQ], BF16, kind="ExternalInput")
    qrep = nc.dram_tensor("qrep", [BPC, T, KQ, 8 * D], F32, kind="ExternalInput")
    kfs = [
        nc.dram_tensor(f"kf{b}", [V * T, D], F32, kind="ExternalInput")
        for b in range(BPC)
    ]
    identb = nc.dram_tensor("identb", [128, 128], BF16, kind="ExternalInput")

    out_o = nc.dram_tensor("out_o", [BPC, T, KQ, D], F32, kind="ExternalOutput")
    tok_o = nc.dram_tensor("tok_o", [BPC, T, KQ, 1], U32, kind="ExternalOutput")
    if DEBUG:
        dbg_idx8 = nc.dram_tensor("dbg_idx8", [128, 8], U32, kind="ExternalOutput")
        dbg_r32 = nc.dram_tensor("dbg_r32", [128, 8], U32, kind="ExternalOutput")
        dbg_cand = nc.dram_tensor("dbg_cand", [128, 8 * D], F32, kind="ExternalOutput")
        dbg_qr = nc.dram_tensor("dbg_qr", [128, 8 * D], F32, kind="ExternalOutput")
        dbg_dots = nc.dram_tensor("dbg_dots", [128, 8], F32, kind="ExternalOutput")
        dbg_m = nc.dram_tensor("dbg_m", [128, 1], F32, kind="ExternalOutput")
        dbg_mask = nc.dram_tensor("dbg_mask", [128, 8], U32, kind="ExternalOutput")
        dbg_sel = nc.dram_tensor("dbg_sel", [128, 8], F32, kind="ExternalOutput")

    with tile.TileContext(nc) as tc:
        from contextlib import ExitStack

        with ExitStack() as ctx:
            const = ctx.enter_context(tc.tile_pool(name="const", bufs=1))
            kpool = ctx.enter_context(tc.tile_pool(name="kpool", bufs=8))
            vpool = ctx.enter_context(tc.tile_pool(name="vpool", bufs=5))
            slabp = ctx.enter_context(tc.tile_pool(name="slabp", bufs=2))
            etp = ctx.enter_context(tc.tile_pool(name="etp", bufs=4))
            small = ctx.enter_context(tc.tile_pool(name="small", bufs=2))
            outp = ctx.enter_context(tc.tile_pool(name="outp", bufs=2))
            ps_qk = ctx.enter_context(tc.tile_pool(name="ps_qk", bufs=1, space="PSUM"))
            ps_tr = ctx.enter_context(tc.tile_pool(name="ps_tr", bufs=1, space="PSUM"))
            ps_av = ctx.enter_context(tc.tile_pool(name="ps_av", bufs=1, space="PSUM"))

            ident_sb = const.tile([128, 128], BF16, tag="ident")
            nc.sync.dma_start(ident_sb[:], identb.ap())
            qtb_sb = []
            for b in range(BPC):
                t_ = const.tile([128, T * KQ], BF16, tag=f"qtb{b}")
                nc.sync.dma_start(t_[:], qtb.ap()[b])
                qtb_sb.append(t_)

            # rotating PSUM buffers, memset once so untouched gap rows stay finite
            psqk_bufs = [ps_qk.tile([128, CW], F32, tag=f"qk{i}", name=f"psqk{i}") for i in range(2)]
            pst_bufs = [ps_tr.tile([128, 112], BF16, tag=f"tr{i}", name=f"pst{i}") for i in range(2)]
            psav_bufs = [ps_av.tile([128, D], F32, tag=f"av{i}", name=f"psav{i}") for i in range(T)]
            for p in psqk_bufs + psav_bufs:
                nc.vector.memset(p[:], 0.0)

            for b in range(BPC):
                # --- load K^T tiles for the 4 t's: [128(h,d), 4096(v)] each
                ktiles = []
                for t in range(T):
                    kt = kpool.tile([128, V // 2], BF16, tag="kt")
                    nc.sync.dma_start(kt[:], ktb.ap()[b, t])
                    ktiles.append(kt)
                # --- V tiles: 4 x [128, 4096] (16 128-v subchunks each)
                vtiles = []
                for vc in range(4):
                    vt = vpool.tile([128, 4096], BF16, tag="vt")
                    nc.sync.dma_start(vt[:], vb.ap()[b, vc])
                    vtiles.append(vt)

                slab = slabp.tile([128, V], BF16, tag="slab")
                sums_part = small.tile([128, NCHUNK], F32, tag="sums_part")

                def emit_qk_exp(c):
                    h = c // 8
                    xo = CW * (c % 8)
                    psqk = psqk_bufs[c % 2]
                    for t in range(T):
                        nc.tensor.matmul(
                            psqk[32 * t:32 * t + KQ, :],
                            qtb_sb[b][64 * h:64 * h + 64, KQ * t:KQ * (t + 1)],
                            ktiles[t][64 * h:64 * h + 64, xo:xo + CW],
                            start=True,
                            stop=True,
                            tile_position=(64 * h, 32 * t),
                        )
                    nc.scalar.activation(
                        slab[0:112, CW * c:CW * (c + 1)],
                        psqk[0:112, :],
                        mybir.ActivationFunctionType.Exp,
                        scale=0.125,
                        accum_out=sums_part[0:112, c:c + 1],
                    )

                def emit_tail(c):
                    for s in range(4 if "av" in FEATURES else 0):
                        g = 4 * c + s
                        pst = pst_bufs[g % 2]
                        nc.tensor.transpose(
                            pst[0:128, 0:112],
                            slab[0:112, CW * c + 128 * s:CW * c + 128 * (s + 1)],
                            ident_sb[0:112, 0:112],
                        )
                        eT = etp.tile([128, 112], BF16, tag="eT")
                        nc.scalar.copy(eT[:], pst[:])
                        vt = vtiles[g // 16]
                        vbase = (g % 16) * T * D
                        for t in range(T):
                            nc.tensor.matmul(
                                psav_bufs[t][32 * t:32 * t + KQ, :],
                                eT[:, 32 * t:32 * t + KQ],
                                vt[:, vbase + D * t:vbase + D * (t + 1)],
                                start=(g == 0),
                                stop=(g == NSUB - 1),
                                tile_position=(0, 32 * t),
                                skip_group_check=True,
                            )

                # software-pipelined by one chunk: exp(c+1) enters the ACT
                # queue before chunk c's eT copies, so the QK/exp chain never
                # stalls behind copies that wait on PE transposes.
                for c in range(NCHUNK):
                    emit_qk_exp(c)
                    if c >= 1:
                        emit_tail(c - 1)
                emit_tail(NCHUNK - 1)

                # --- softmax denominator and normalization
                sums = small.tile([128, 1], F32, tag="sums")
                nc.vector.tensor_reduce(
                    sums[0:112], sums_part[0:112, :],
                    axis=mybir.AxisListType.X, op=mybir.AluOpType.add,
                )
                recip = small.tile([128, 1], F32, tag="recip")
                nc.vector.reciprocal(recip[0:112], sums[0:112])
                outsb = outp.tile([128, D], F32, tag="outsb")
                for t in range(T):
                    nc.vector.tensor_scalar(
                        outsb[32 * t:32 * t + KQ, :],
                        psav_bufs[t][32 * t:32 * t + KQ, :],
                        recip[32 * t:32 * t + KQ], None,
                        op0=mybir.AluOpType.mult,
                    )
                for t in range(T):
                    nc.sync.dma_start(
                        out_o.ap()[b, t], outsb[32 * t:32 * t + KQ, :]
                    )

                # --- argmax candidates (bf16 slab) + exact fp32 rescore
                if "argmax" not in FEATURES:
                    continue
                top8 = small.tile([128, 8], BF16, tag="top8")
                nc.vector.max(top8[0:112, :], slab[0:112, :])
                idx8 = small.tile([128, 8], U32, tag="idx8")
                nc.vector.max_index(idx8[0:112, :], top8[0:112, :], slab[0:112, :])

                if "rescore" not in FEATURES:
                    continue
                # gather row indices r = 4*v + t into k[b] viewed [V*T, D]
                r32 = small.tile([128, 8], U32, tag="r32")
                cand = outp.tile([128, 8 * D], F32, tag="cand")
                qr_sb = outp.tile([128, 8 * D], F32, tag="qr_sb")
                nc.vector.memset(r32[:], 0)
                nc.vector.memset(cand[:], 0.0)
                nc.vector.memset(qr_sb[:], 0.0)
                for t in range(T):
                    nc.vector.tensor_scalar(
                        r32[32 * t:32 * t + KQ, :], idx8[32 * t:32 * t + KQ, :],
                        4, t, op0=mybir.AluOpType.mult, op1=mybir.AluOpType.add,
                    )
                    nc.sync.dma_start(
                        qr_sb[32 * t:32 * t + KQ, :], qrep.ap()[b, t]
                    )
                if "gather" in FEATURES:
                    for cc in range(8):
                        nc.gpsimd.indirect_dma_start(
                            out=cand[0:112, D * cc:D * (cc + 1)],
                            out_offset=None,
                            in_=kfs[b].ap(),
                            in_offset=bass.IndirectOffsetOnAxis(
                                ap=r32[0:112, cc:cc + 1], axis=0
                            ),
                        )
                prod = outp.tile([128, 8 * D], F32, tag="prod")
                nc.vector.tensor_tensor(
                    prod[0:112, :], cand[0:112, :], qr_sb[0:112, :],
                    op=mybir.AluOpType.mult,
                )
                dots = small.tile([128, 8], F32, tag="dots")
                nc.vector.tensor_reduce(
                    dots[0:112, :],
                    prod[0:112, :].rearrange("p (c d) -> p c d", d=D),
                    axis=mybir.AxisListType.X, op=mybir.AluOpType.add,
                )
                m = small.tile([128, 1], F32, tag="m")
                nc.vector.tensor_reduce(
                    m[0:112], dots[0:112, :],
                    axis=mybir.AxisListType.X, op=mybir.AluOpType.max,
                )
                mask = small.tile([128, 8], U32, tag="mask")
                nc.vector.tensor_tensor(
                    mask[0:112, :], dots[0:112, :],
                    m[0:112].to_broadcast([112, 8]),
                    op=mybir.AluOpType.is_ge,
                )
                idxf = small.tile([128, 8], F32, tag="idxf")
                nc.vector.tensor_copy(idxf[0:112, :], idx8[0:112, :])
                sel = small.tile([128, 8], F32, tag="sel")
                nc.vector.memset(sel[:], 16384.0)
                nc.vector.copy_predicated(
                    sel[0:112, :], mask[0:112, :], idxf[0:112, :]
                )
                if DEBUG and b == 0:
                    nc.sync.dma_start(dbg_idx8.ap()[0:112], idx8[0:112, :])
                    nc.sync.dma_start(dbg_r32.ap()[0:112], r32[0:112, :])
                    nc.sync.dma_start(dbg_cand.ap()[0:112], cand[0:112, :])
                    nc.sync.dma_start(dbg_qr.ap()[0:112], qr_sb[0:112, :])
                    nc.sync.dma_start(dbg_dots.ap()[0:112], dots[0:112, :])
                    nc.sync.dma_start(dbg_m.ap()[0:112], m[0:112, :])
                    nc.sync.dma_start(dbg_mask.ap()[0:112], mask[0:112, :])
                    nc.sync.dma_start(dbg_sel.ap()[0:112], sel[0:112, :])
                tokf = small.tile([128, 1], F32, tag="tokf")
                nc.vector.tensor_reduce(
                    tokf[0:112], sel[0:112, :],
                    axis=mybir.AxisListType.X, op=mybir.AluOpType.min,
                )
                toku = small.tile([128, 1], U32, tag="toku")
                nc.vector.tensor_copy(toku[0:112], tokf[0:112])
                for t in range(T):
                    nc.sync.dma_start(
                        tok_o.ap()[b, t], toku[32 * t:32 * t + KQ, :]
                    )

    nc.compile()
    return nc


def _prep_in_maps(q, k, v):
    ident = np.eye(128, dtype=ml_dtypes.bfloat16)
    in_maps = []
    for c in range(NCORES):
        sl = slice(BPC * c, BPC * (c + 1))
        qc, kc, vc = q[sl], k[sl], v[sl]
        # ktb_pre[b, t, 64h+d, x] = k[b, 4096h+x, t, d]
        ktb = (
            kc.transpose(0, 2, 3, 1)                      # (b, t, d, v)
            .reshape(BPC, T, D, 2, V // 2)
            .transpose(0, 1, 3, 2, 4)                     # (b, t, h, d, x)
            .reshape(BPC, T, 128, V // 2)
        )
        ktb = np.ascontiguousarray(ktb).astype(ml_dtypes.bfloat16)
        # vb_pre[b, vc, p, s*256+x] = v[b, 2048*vc + 128*s + p, t, d]
        vbb = (
            vc.reshape(BPC, 4, 16, 128, T * D)            # (b, vc, s, p, x)
            .transpose(0, 1, 3, 2, 4)                     # (b, vc, p, s, x)
            .reshape(BPC, 4, 128, 4096)
        )
        vbb = np.ascontiguousarray(vbb).astype(ml_dtypes.bfloat16)
        # qtb[b, 64h+d, 16t+j] = q[b, j, t, d], duplicated across h
        qq = np.ascontiguousarray(qc.transpose(0, 3, 2, 1)).reshape(BPC, D, T * KQ)
        qtb = np.concatenate([qq, qq], axis=1).astype(ml_dtypes.bfloat16)
        # qrep[b, t, j, 8*D] = q[b, j, t, :] tiled 8x
        qr = np.ascontiguousarray(qc.transpose(0, 2, 1, 3))          # (b, t, j, d)
        qrep = np.broadcast_to(
            qr[:, :, :, None, :], (BPC, T, KQ, 8, D)
        ).reshape(BPC, T, KQ, 8 * D).copy()
        im = dict(
            ktb=ktb, vb=vbb, qtb=qtb, qrep=qrep,
            identb=ident,
        )
        for b in range(BPC):
            im[f"kf{b}"] = np.ascontiguousarray(kc[b].reshape(V * T, D))
        in_maps.append(im)
    return in_maps


def _postprocess(res):
    out = np.empty((BS, KQ, T, D), dtype=np.float32)
    tokens = np.empty((BS, KQ, T), dtype=np.int32)
    for c in range(NCORES):
        r = res.results[c]
        oo = r["out_o"]            # (BPC, T, KQ, D)
        tt = r["tok_o"][..., 0]    # (BPC, T, KQ)
        for b in range(BPC):
            out[BPC * c + b] = oo[b].transpose(1, 0, 2)
            tokens[BPC * c + b] = tt[b].astype(np.int64).T.astype(np.int32)
    return out, tokens


def kernel(q: np.ndarray, k: np.ndarray, v: np.ndarray):
    q = np.asarray(q, dtype=np.float32)
    k = np.asarray(k, dtype=np.float32)
    v = np.asarray(v, dtype=np.float32)
    if "nc" not in _CACHED:
        _CACHED["nc"] = _build()
    in_maps = _prep_in_maps(q, k, v)
    res = run_bass_kernel_spmd(_CACHED["nc"], in_maps, core_ids=list(range(NCORES)))
    return _postprocess(res)


def run_traced(q: np.ndarray, k: np.ndarray, v: np.ndarray):
    """Like kernel() but with NTFF tracing; returns BassKernelResults."""
    q = np.asarray(q, dtype=np.float32)
    k = np.asarray(k, dtype=np.float32)
    v = np.asarray(v, dtype=np.float32)
    if "nc" not in _CACHED:
        _CACHED["nc"] = _build()
    in_maps = _prep_in_maps(q, k, v)
    return run_bass_kernel_spmd(
        _CACHED["nc"], in_maps, core_ids=list(range(NCORES)), trace=True
    )


# revision 25
# speedup vs baseline: 1.0080x; 1.0080x over previous
"""BlockAttention Trainium2 kernel.

Full inputs: q (32,16,4,64) f32, k/v (32,8192,4,64) f32.
Outputs: (out (32,16,4,64) f32, tokens (32,16,4) int32).

Sharding: data-parallel over batch, 4 batches per core x 8 cores.

Per-core device algorithm, per batch-group b (the 4 t's of one batch are
processed together, packed on PSUM partition groups {0,32,64,96}):
  - QK^T in bf16 on TensorE (Q stationary [64d,16kq], K^T streamed in 512-v
    chunks), logits*1/8 -> exp via ScalarE (fp32 in PSUM -> bf16 slab in SBUF,
    fused per-partition row-sum accumulation = softmax denominator).
  - PE-transposes of 128-v exp chunks -> attn^T tiles -> bf16 AV matmuls
    accumulated over the whole vocab in PSUM; normalized by reciprocal sum.
  - tokens: top-8 candidates per row via max8/max_index on the exp slab
    (exp is monotonic in the logits), then an *exact fp32 rescore*: the 8
    candidate k-vectors per row are gathered from the untouched fp32 k via
    indirect DMA and re-dotted with q on VectorE; argmax with min-index
    tie-break (matches jnp.argmax first-occurrence).
"""

import sys

import numpy as np

try:
    import concourse.bass as bass  # noqa: F401
except ImportError:  # pragma: no cover
    sys.path.insert(0, "/opt/trn_rl_repo")

import ml_dtypes

import concourse.bass as bass
import concourse.tile as tile
from concourse import bacc, mybir
from concourse.bass_utils import run_bass_kernel_spmd

F32 = mybir.dt.float32
BF16 = mybir.dt.bfloat16
U32 = mybir.dt.uint32

BS, KQ, T, D, V = 32, 16, 4, 64, 8192
NCORES = 8
BPC = BS // NCORES          # batches per core = 4
NCHUNK = 16                 # 512-v chunks per (b,t)
CW = 512                    # chunk width (PSUM bank in fp32)
NSUB = V // 128             # 64 128-v subchunks per (b,t)

_CACHED = {}
FEATURES = {"av", "argmax", "rescore", "gather"}
DEBUG = False


def _build():
    nc = bacc.Bacc("TRN2", num_devices=NCORES)

    ktb = nc.dram_tensor("ktb", [BPC, T, 128, V // 2], BF16, kind="ExternalInput")
    vb = nc.dram_tensor("vb", [BPC, 4, 128, 4096], BF16, kind="ExternalInput")
    qtb = nc.dram_tensor("qtb", [BPC, 128, T * KQ], BF16, kind="ExternalInput")
    qrep = nc.dram_tensor("qrep", [BPC, T, KQ, 8 * D], F32, kind="ExternalInput")
    kfs = [
        nc.dram_tensor(f"kf{b}", [V * T, D], F32, kind="ExternalInput")
        for b in range(BPC)
    ]
    identb = nc.dram_tensor("identb", [128, 128], BF16, kind="ExternalInput")

    out_o = nc.dram_tensor("out_o", [BPC, T, KQ, D], F32, kind="ExternalOutput")
    tok_o = nc.dram_tensor("tok_o", [BPC, T, KQ, 1], U32, kind="ExternalOutput")
    if DEBUG:
        dbg_idx8 = nc.dram_tensor("dbg_idx8", [128, 8], U32, kind="ExternalOutput")
        dbg_r32 = nc.dram_tensor("dbg_r32", [128, 8], U32, kind="ExternalOutput")
        dbg_cand = nc.dram_tensor("dbg_cand", [128, 8 * D], F32, kind="ExternalOutput")
        dbg_qr = nc.dram_tensor("dbg_qr", [128, 8 * D], F32, kind="ExternalOutput")
        dbg_dots = nc.dram_tensor("dbg_dots", [128, 8], F32, kind="ExternalOutput")
        dbg_m = nc.dram_tensor("dbg_m", [128, 1], F32, kind="ExternalOutput")
        dbg_mask = nc.dram_tensor("dbg_mask", [128, 8], U32, kind="ExternalOutput")
        dbg_sel = nc.dram_tensor("dbg_sel", [128, 8], F32, kind="ExternalOutput")

    with tile.TileContext(nc) as tc:
        from contextlib import ExitStack

        with ExitStack() as ctx:
            const = ctx.enter_context(tc.tile_pool(name="const", bufs=1))
            kpool = ctx.enter_context(tc.tile_pool(name="kpool", bufs=8))
            vpool = ctx.enter_context(tc.tile_pool(name="vpool", bufs=5))
            slabp = ctx.enter_context(tc.tile_pool(name="slabp", bufs=2))
            etp = ctx.enter_context(tc.tile_pool(name="etp", bufs=4))
            small = ctx.enter_context(tc.tile_pool(name="small", bufs=2))
            outp = ctx.enter_context(tc.tile_pool(name="outp", bufs=2))
            ps_qk = ctx.enter_context(tc.tile_pool(name="ps_qk", bufs=1, space="PSUM"))
            ps_tr = ctx.enter_context(tc.tile_pool(name="ps_tr", bufs=1, space="PSUM"))
            ps_av = ctx.enter_context(tc.tile_pool(name="ps_av", bufs=1, space="PSUM"))

            ident_sb = const.tile([128, 128], BF16, tag="ident")
            nc.sync.dma_start(ident_sb[:], identb.ap())
            qtb_sb = []
            for b in range(BPC):
                t_ = const.tile([128, T * KQ], BF16, tag=f"qtb{b}")
                nc.sync.dma_start(t_[:], qtb.ap()[b])
                qtb_sb.append(t_)

            # rotating PSUM buffers, memset once so untouched gap rows stay finite
            psqk_bufs = [ps_qk.tile([128, CW], F32, tag=f"qk{i}", name=f"psqk{i}") for i in range(2)]
            pst_bufs = [ps_tr.tile([128, 112], BF16, tag=f"tr{i}", name=f"pst{i}") for i in range(2)]
            psav_bufs = [ps_av.tile([128, D], F32, tag=f"av{i}", name=f"psav{i}") for i in range(T)]
            for p in psqk_bufs + psav_bufs:
                nc.vector.memset(p[:], 0.0)

            for b in range(BPC):
                # --- load K^T tiles for the 4 t's: [128(h,d), 4096(v)] each
                ktiles = []
                for t in range(T):
                    kt = kpool.tile([128, V // 2], BF16, tag="kt")
                    nc.sync.dma_start(kt[:], ktb.ap()[b, t])
                    ktiles.append(kt)
                # --- V tiles: 4 x [128, 4096] (16 128-v subchunks each)
                vtiles = []
                for vc in range(4):
                    vt = vpool.tile([128, 4096], BF16, tag="vt")
                    nc.sync.dma_start(vt[:], vb.ap()[b, vc])
                    vtiles.append(vt)

                slab = slabp.tile([128, V], BF16, tag="slab")
                sums_part = small.tile([128, NCHUNK], F32, tag="sums_part")

                def emit_qk_exp(c):
                    h = c // 8
                    xo = CW * (c % 8)
                    psqk = psqk_bufs[c % 2]
                    for t in range(T):
                        nc.tensor.matmul(
                            psqk[32 * t:32 * t + KQ, :],
                            qtb_sb[b][64 * h:64 * h + 64, KQ * t:KQ * (t + 1)],
                            ktiles[t][64 * h:64 * h + 64, xo:xo + CW],
                            start=True,
                            stop=True,
                            tile_position=(64 * h, 32 * t),
                        )
                    nc.scalar.activation(
                        slab[0:112, CW * c:CW * (c + 1)],
                        psqk[0:112, :],
                        mybir.ActivationFunctionType.Exp,
                        scale=0.125,
                        accum_out=sums_part[0:112, c:c + 1],
                    )

                def emit_tail(c):
                    for s in range(4 if "av" in FEATURES else 0):
                        g = 4 * c + s
                        pst = pst_bufs[g % 2]
                        nc.tensor.transpose(
                            pst[0:128, 0:112],
                            slab[0:112, CW * c + 128 * s:CW * c + 128 * (s + 1)],
                            ident_sb[0:112, 0:112],
                        )
                        eT = etp.tile([128, 112], BF16, tag="eT")
                        nc.scalar.copy(eT[:], pst[:])
                        vt = vtiles[g // 16]
                        vbase = (g % 16) * T * D
                        for t in range(T):
                            nc.tensor.matmul(
                                psav_bufs[t][32 * t:32 * t + KQ, :],
                                eT[:, 32 * t:32 * t + KQ],
                                vt[:, vbase + D * t:vbase + D * (t + 1)],
                                start=(g == 0),
                                stop=(g == NSUB - 1),
                                tile_position=(0, 32 * t),
                                skip_group_check=True,
                            )

                # software-pipelined by one chunk: exp(c+1) enters the ACT
                # queue before chunk c's eT copies, so the QK/exp chain never
                # stalls behind copies that wait on PE transposes.
                for c in range(NCHUNK):
                    emit_qk_exp(c)
                    if c >= 1:
                        emit_tail(c - 1)
                emit_tail(NCHUNK - 1)

                # --- softmax denominator and normalization
                sums = small.tile([128, 1], F32, tag="sums")
                nc.vector.tensor_reduce(
                    sums[0:112], sums_part[0:112, :],
                    axis=mybir.AxisListType.X, op=mybir.AluOpType.add,
                )
                recip = small.tile([128, 1], F32, tag="recip")
                nc.vector.reciprocal(recip[0:112], sums[0:112])
                outsb = outp.tile([128, D], F32, tag="outsb")
                for t in range(T):
                    nc.vector.tensor_scalar(
                        outsb[32 * t:32 * t + KQ, :],
                        psav_bufs[t][32 * t:32 * t + KQ, :],
                        recip[32 * t:32 * t + KQ], None,
                        op0=mybir.AluOpType.mult,
                    )
                for t in range(T):
                    nc.gpsimd.dma_start(
                        out_o.ap()[b, t], outsb[32 * t:32 * t + KQ, :]
                    )

                # --- argmax candidates (bf16 slab) + exact fp32 rescore
                if "argmax" not in FEATURES:
                    continue
                top8 = small.tile([128, 8], BF16, tag="top8")
                nc.vector.max(top8[0:112, :], slab[0:112, :])
                idx8 = small.tile([128, 8], U32, tag="idx8")
                nc.vector.max_index(idx8[0:112, :], top8[0:112, :], slab[0:112, :])

                if "rescore" not in FEATURES:
                    continue
                # gather row indices r = 4*v + t into k[b] viewed [V*T, D]
                r32 = small.tile([128, 8], U32, tag="r32")
                cand = outp.tile([128, 8 * D], F32, tag="cand")
                qr_sb = outp.tile([128, 8 * D], F32, tag="qr_sb")
                nc.vector.memset(r32[:], 0)
                nc.vector.memset(cand[:], 0.0)
                nc.vector.memset(qr_sb[:], 0.0)
                for t in range(T):
                    nc.vector.tensor_scalar(
                        r32[32 * t:32 * t + KQ, :], idx8[32 * t:32 * t + KQ, :],
                        4, t, op0=mybir.AluOpType.mult, op1=mybir.AluOpType.add,
                    )
                    nc.sync.dma_start(
                        qr_sb[32 * t:32 * t + KQ, :], qrep.ap()[b, t]
                    )
                if "gather" in FEATURES:
                    for cc in range(8):
                        nc.gpsimd.indirect_dma_start(
                            out=cand[0:112, D * cc:D * (cc + 1)],
                            out_offset=None,
                            in_=kfs[b].ap(),
                            in_offset=bass.IndirectOffsetOnAxis(
                                ap=r32[0:112, cc:cc + 1], axis=0
                            ),
                        )
                prod = outp.tile([128, 8 * D], F32, tag="prod")
                nc.vector.tensor_tensor(
                    prod[0:112, :], cand[0:112, :], qr_sb[0:112, :],
                    op=mybir.AluOpType.mult,
                )
                dots = small.tile([128, 8], F32, tag="dots")
                nc.vector.tensor_reduce(
                    dots[0:112, :],
                    prod[0:112, :].rearrange("p (c d) -> p c d", d=D),
                    axis=mybir.AxisListType.X, op=mybir.AluOpType.add,
                )
                m = small.tile([128, 1], F32, tag="m")
                nc.vector.tensor_reduce(
                    m[0:112], dots[0:112, :],
                    axis=mybir.AxisListType.X, op=mybir.AluOpType.max,
                )
                mask = small.tile([128, 8], U32, tag="mask")
                nc.vector.tensor_tensor(
                    mask[0:112, :], dots[0:112, :],
                    m[0:112].to_broadcast([112, 8]),
                    op=mybir.AluOpType.is_ge,
                )
                idxf = small.tile([128, 8], F32, tag="idxf")
                nc.vector.tensor_copy(idxf[0:112, :], idx8[0:112, :])
                sel = small.tile([128, 8], F32, tag="sel")
                nc.vector.memset(sel[:], 16384.0)
                nc.vector.copy_predicated(
                    sel[0:112, :], mask[0:112, :], idxf[0:112, :]
                )
                if DEBUG and b == 0:
                    nc.sync.dma_start(dbg_idx8.ap()[0:112], idx8[0:112, :])
                    nc.sync.dma_start(dbg_r32.ap()[0:112], r32[0:112, :])
                    nc.sync.dma_start(dbg_cand.ap()[0:112], cand[0:112, :])
                    nc.sync.dma_start(dbg_qr.ap()[0:112], qr_sb[0:112, :])
                    nc.sync.dma_start(dbg_dots.ap()[0:112], dots[0:112, :])
                    nc.sync.dma_start(dbg_m.ap()[0:112], m[0:112, :])
                    nc.sync.dma_start(dbg_mask.ap()[0:112], mask[0:112, :])
                    nc.sync.dma_start(dbg_sel.ap()[0:112], sel[0:112, :])
                tokf = small.tile([128, 1], F32, tag="tokf")
                nc.vector.tensor_reduce(
                    tokf[0:112], sel[0:112, :],
                    axis=mybir.AxisListType.X, op=mybir.AluOpType.min,
                )
                toku = small.tile([128, 1], U32, tag="toku")
                nc.vector.tensor_copy(toku[0:112], tokf[0:112])
                for t in range(T):
                    nc.gpsimd.dma_start(
                        tok_o.ap()[b, t], toku[32 * t:32 * t + KQ, :]
                    )

    nc.compile()
    return nc


def _prep_in_maps(q, k, v):
    ident = np.eye(128, dtype=ml_dtypes.bfloat16)
    in_maps = []
    for c in range(NCORES):
        sl = slice(BPC * c, BPC * (c + 1))
        qc, kc, vc = q[sl], k[sl], v[sl]
        # ktb_pre[b, t, 64h+d, x] = k[b, 4096h+x, t, d]
        ktb = (
            kc.transpose(0, 2, 3, 1)                      # (b, t, d, v)
            .reshape(BPC, T, D, 2, V // 2)
            .transpose(0, 1, 3, 2, 4)                     # (b, t, h, d, x)
            .reshape(BPC, T, 128, V // 2)
        )
        ktb = np.ascontiguousarray(ktb).astype(ml_dtypes.bfloat16)
        # vb_pre[b, vc, p, s*256+x] = v[b, 2048*vc + 128*s + p, t, d]
        vbb = (
            vc.reshape(BPC, 4, 16, 128, T * D)            # (b, vc, s, p, x)
            .transpose(0, 1, 3, 2, 4)                     # (b, vc, p, s, x)
            .reshape(BPC, 4, 128, 4096)
        )
        vbb = np.ascontiguousarray(vbb).astype(ml_dtypes.bfloat16)
        # qtb[b, 64h+d, 16t+j] = q[b, j, t, d], duplicated across h
        qq = np.ascontiguousarray(qc.transpose(0, 3, 2, 1)).reshape(BPC, D, T * KQ)
        qtb = np.concatenate([qq, qq], axis=1).astype(ml_dtypes.bfloat16)
        # qrep[b, t, j, 8*D] = q[b, j, t, :] tiled 8x
        qr = np.ascontiguousarray(qc.transpose(0, 2, 1, 3))          # (b, t, j, d)
        qrep = np.broadcast_to(
            qr[:, :, :, None, :], (BPC, T, KQ, 8, D)
        ).reshape(BPC, T, KQ, 8 * D).copy()
        im = dict(
            ktb=ktb, vb=vbb, qtb=qtb, qrep=qrep,
            identb=ident,
        )
        for b in range(BPC):
            im[f"kf{b}"] = np.ascontiguousarray(kc[b].reshape(V * T, D))
        in_maps.append(im)
    return in_maps


def _postprocess(res):
    out = np.empty((BS, KQ, T, D), dtype=np.float32)
    tokens = np.empty((BS, KQ, T), dtype=np.int32)
    for c in range(NCORES):
        r = res.results[c]
        oo = r["out_o"]            # (BPC, T, KQ, D)
        tt = r["tok_o"][..., 0]    # (BPC, T, KQ)
        for b in range(BPC):
            out[BPC * c + b] = oo[b].transpose(1, 0, 2)
            tokens[BPC * c + b] = tt[b].astype(np.int64).T.astype(np.int32)
    return out, tokens


def kernel(q: np.ndarray, k: np.ndarray, v: np.ndarray):
    q = np.asarray(q, dtype=np.float32)
    k = np.asarray(k, dtype=np.float32)
    v = np.asarray(v, dtype=np.float32)
    if "nc" not in _CACHED:
        _CACHED["nc"] = _build()
    in_maps = _prep_in_maps(q, k, v)
    res = run_bass_kernel_spmd(_CACHED["nc"], in_maps, core_ids=list(range(NCORES)))
    return _postprocess(res)


def run_traced(q: np.ndarray, k: np.ndarray, v: np.ndarray):
    """Like kernel() but with NTFF tracing; returns BassKernelResults."""
    q = np.asarray(q, dtype=np.float32)
    k = np.asarray(k, dtype=np.float32)
    v = np.asarray(v, dtype=np.float32)
    if "nc" not in _CACHED:
        _CACHED["nc"] = _build()
    in_maps = _prep_in_maps(q, k, v)
    return run_bass_kernel_spmd(
        _CACHED["nc"], in_maps, core_ids=list(range(NCORES)), trace=True
    )


# revision 26
# speedup vs baseline: 1.1051x; 1.0963x over previous
"""BlockAttention Trainium2 kernel.

Full inputs: q (32,16,4,64) f32, k/v (32,8192,4,64) f32.
Outputs: (out (32,16,4,64) f32, tokens (32,16,4) int32).

Sharding: data-parallel over batch, 4 batches per core x 8 cores.

Per-core device algorithm, per batch-group b (the 4 t's of one batch are
processed together, packed on PSUM partition groups {0,32,64,96}):
  - QK^T in bf16 on TensorE (Q stationary [64d,16kq], K^T streamed in 512-v
    chunks), logits*1/8 -> exp via ScalarE (fp32 in PSUM -> bf16 slab in SBUF,
    fused per-partition row-sum accumulation = softmax denominator).
  - PE-transposes of 128-v exp chunks -> attn^T tiles -> bf16 AV matmuls
    accumulated over the whole vocab in PSUM; normalized by reciprocal sum.
  - tokens: top-8 candidates per row via max8/max_index on the exp slab
    (exp is monotonic in the logits), then an *exact fp32 rescore*: the 8
    candidate k-vectors per row are gathered from the untouched fp32 k via
    indirect DMA and re-dotted with q on VectorE; argmax with min-index
    tie-break (matches jnp.argmax first-occurrence).
"""

import sys

import numpy as np

try:
    import concourse.bass as bass  # noqa: F401
except ImportError:  # pragma: no cover
    sys.path.insert(0, "/opt/trn_rl_repo")

import ml_dtypes

import concourse.bass as bass
import concourse.tile as tile
from concourse import bacc, mybir
from concourse.bass_utils import run_bass_kernel_spmd

F32 = mybir.dt.float32
BF16 = mybir.dt.bfloat16
U32 = mybir.dt.uint32

BS, KQ, T, D, V = 32, 16, 4, 64, 8192
NCORES = 8
BPC = BS // NCORES          # batches per core = 4
NCHUNK = 16                 # 512-v chunks per (b,t)
CW = 512                    # chunk width (PSUM bank in fp32)
NSUB = V // 128             # 64 128-v subchunks per (b,t)

_CACHED = {}
FEATURES = {"av", "argmax", "rescore", "gather"}
DEBUG = False


def _build():
    nc = bacc.Bacc("TRN2", num_devices=NCORES)

    ktb = nc.dram_tensor("ktb", [BPC, T, 128, V // 2], BF16, kind="ExternalInput")
    vb = nc.dram_tensor("vb", [BPC, 4, 128, 4096], BF16, kind="ExternalInput")
    qtb = nc.dram_tensor("qtb", [BPC, 128, T * KQ], BF16, kind="ExternalInput")
    qrep = nc.dram_tensor("qrep", [BPC, T, KQ, 8 * D], F32, kind="ExternalInput")
    kfs = [
        nc.dram_tensor(f"kf{b}", [V * T, D], F32, kind="ExternalInput")
        for b in range(BPC)
    ]
    identb = nc.dram_tensor("identb", [128, 128], BF16, kind="ExternalInput")

    out_o = nc.dram_tensor("out_o", [BPC, T, KQ, D], F32, kind="ExternalOutput")
    tok_o = nc.dram_tensor("tok_o", [BPC, T, KQ, 1], U32, kind="ExternalOutput")
    if DEBUG:
        dbg_idx8 = nc.dram_tensor("dbg_idx8", [128, 8], U32, kind="ExternalOutput")
        dbg_r32 = nc.dram_tensor("dbg_r32", [128, 8], U32, kind="ExternalOutput")
        dbg_cand = nc.dram_tensor("dbg_cand", [128, 8 * D], F32, kind="ExternalOutput")
        dbg_qr = nc.dram_tensor("dbg_qr", [128, 8 * D], F32, kind="ExternalOutput")
        dbg_dots = nc.dram_tensor("dbg_dots", [128, 8], F32, kind="ExternalOutput")
        dbg_m = nc.dram_tensor("dbg_m", [128, 1], F32, kind="ExternalOutput")
        dbg_mask = nc.dram_tensor("dbg_mask", [128, 8], U32, kind="ExternalOutput")
        dbg_sel = nc.dram_tensor("dbg_sel", [128, 8], F32, kind="ExternalOutput")

    with tile.TileContext(nc) as tc:
        from contextlib import ExitStack

        with ExitStack() as ctx:
            const = ctx.enter_context(tc.tile_pool(name="const", bufs=1))
            kpool = ctx.enter_context(tc.tile_pool(name="kpool", bufs=8))
            vpool = ctx.enter_context(tc.tile_pool(name="vpool", bufs=5))
            slabp = ctx.enter_context(tc.tile_pool(name="slabp", bufs=2))
            etp = ctx.enter_context(tc.tile_pool(name="etp", bufs=4))
            small = ctx.enter_context(tc.tile_pool(name="small", bufs=2))
            outp = ctx.enter_context(tc.tile_pool(name="outp", bufs=2))
            ps_qk = ctx.enter_context(tc.tile_pool(name="ps_qk", bufs=1, space="PSUM"))
            ps_tr = ctx.enter_context(tc.tile_pool(name="ps_tr", bufs=1, space="PSUM"))
            ps_av = ctx.enter_context(tc.tile_pool(name="ps_av", bufs=1, space="PSUM"))

            ident_sb = const.tile([128, 128], BF16, tag="ident")
            nc.sync.dma_start(ident_sb[:], identb.ap())
            qtb_sb = []
            for b in range(BPC):
                t_ = const.tile([128, T * KQ], BF16, tag=f"qtb{b}")
                nc.sync.dma_start(t_[:], qtb.ap()[b])
                qtb_sb.append(t_)

            # rotating PSUM buffers, memset once so untouched gap rows stay finite
            psqk_bufs = [ps_qk.tile([128, CW], F32, tag=f"qk{i}", name=f"psqk{i}") for i in range(2)]
            pst_bufs = [ps_tr.tile([128, 112], BF16, tag=f"tr{i}", name=f"pst{i}") for i in range(2)]
            psav_bufs = [ps_av.tile([128, D], F32, tag=f"av{i}", name=f"psav{i}") for i in range(T)]
            for p in psqk_bufs + psav_bufs:
                nc.vector.memset(p[:], 0.0)

            for b in range(BPC):
                # --- load K^T tiles for the 4 t's: [128(h,d), 4096(v)] each
                ktiles = []
                for t in range(T):
                    kt = kpool.tile([128, V // 2], BF16, tag="kt")
                    nc.sync.dma_start(kt[:], ktb.ap()[b, t])
                    ktiles.append(kt)
                # --- V tiles: 4 x [128, 4096] (16 128-v subchunks each)
                vtiles = []
                for vc in range(4):
                    vt = vpool.tile([128, 4096], BF16, tag="vt")
                    nc.sync.dma_start(vt[:], vb.ap()[b, vc])
                    vtiles.append(vt)

                slab = slabp.tile([128, V], BF16, tag="slab")
                sums_part = small.tile([128, NCHUNK], F32, tag="sums_part")

                def emit_qk_exp(c):
                    h = c // 8
                    xo = CW * (c % 8)
                    psqk = psqk_bufs[c % 2]
                    # priority-0: once ready, the 4 QK matmuls run back-to-back
                    # on PE (pipelined + HAM-warm) ahead of pending tail work,
                    # and exp jumps ACT's copy queue.
                    with tc.high_priority():
                        for t in range(T):
                            nc.tensor.matmul(
                                psqk[32 * t:32 * t + KQ, :],
                                qtb_sb[b][64 * h:64 * h + 64, KQ * t:KQ * (t + 1)],
                                ktiles[t][64 * h:64 * h + 64, xo:xo + CW],
                                start=True,
                                stop=True,
                                tile_position=(64 * h, 32 * t),
                            )
                        nc.scalar.activation(
                            slab[0:112, CW * c:CW * (c + 1)],
                            psqk[0:112, :],
                            mybir.ActivationFunctionType.Exp,
                            scale=0.125,
                            accum_out=sums_part[0:112, c:c + 1],
                        )

                def emit_tail(c):
                    for s in range(4 if "av" in FEATURES else 0):
                        g = 4 * c + s
                        pst = pst_bufs[g % 2]
                        nc.tensor.transpose(
                            pst[0:128, 0:112],
                            slab[0:112, CW * c + 128 * s:CW * c + 128 * (s + 1)],
                            ident_sb[0:112, 0:112],
                        )
                        eT = etp.tile([128, 112], BF16, tag="eT")
                        nc.scalar.copy(eT[:], pst[:])
                        vt = vtiles[g // 16]
                        vbase = (g % 16) * T * D
                        for t in range(T):
                            nc.tensor.matmul(
                                psav_bufs[t][32 * t:32 * t + KQ, :],
                                eT[:, 32 * t:32 * t + KQ],
                                vt[:, vbase + D * t:vbase + D * (t + 1)],
                                start=(g == 0),
                                stop=(g == NSUB - 1),
                                tile_position=(0, 32 * t),
                                skip_group_check=True,
                            )

                # software-pipelined by one chunk: exp(c+1) enters the ACT
                # queue before chunk c's eT copies, so the QK/exp chain never
                # stalls behind copies that wait on PE transposes.
                for c in range(NCHUNK):
                    emit_qk_exp(c)
                    if c >= 1:
                        emit_tail(c - 1)
                emit_tail(NCHUNK - 1)

                # --- softmax denominator and normalization
                sums = small.tile([128, 1], F32, tag="sums")
                nc.vector.tensor_reduce(
                    sums[0:112], sums_part[0:112, :],
                    axis=mybir.AxisListType.X, op=mybir.AluOpType.add,
                )
                recip = small.tile([128, 1], F32, tag="recip")
                nc.vector.reciprocal(recip[0:112], sums[0:112])
                outsb = outp.tile([128, D], F32, tag="outsb")
                for t in range(T):
                    nc.vector.tensor_scalar(
                        outsb[32 * t:32 * t + KQ, :],
                        psav_bufs[t][32 * t:32 * t + KQ, :],
                        recip[32 * t:32 * t + KQ], None,
                        op0=mybir.AluOpType.mult,
                    )
                for t in range(T):
                    nc.gpsimd.dma_start(
                        out_o.ap()[b, t], outsb[32 * t:32 * t + KQ, :]
                    )

                # --- argmax candidates (bf16 slab) + exact fp32 rescore
                if "argmax" not in FEATURES:
                    continue
                top8 = small.tile([128, 8], BF16, tag="top8")
                nc.vector.max(top8[0:112, :], slab[0:112, :])
                idx8 = small.tile([128, 8], U32, tag="idx8")
                nc.vector.max_index(idx8[0:112, :], top8[0:112, :], slab[0:112, :])

                if "rescore" not in FEATURES:
                    continue
                # gather row indices r = 4*v + t into k[b] viewed [V*T, D]
                r32 = small.tile([128, 8], U32, tag="r32")
                cand = outp.tile([128, 8 * D], F32, tag="cand")
                qr_sb = outp.tile([128, 8 * D], F32, tag="qr_sb")
                nc.vector.memset(r32[:], 0)
                nc.vector.memset(cand[:], 0.0)
                nc.vector.memset(qr_sb[:], 0.0)
                for t in range(T):
                    nc.vector.tensor_scalar(
                        r32[32 * t:32 * t + KQ, :], idx8[32 * t:32 * t + KQ, :],
                        4, t, op0=mybir.AluOpType.mult, op1=mybir.AluOpType.add,
                    )
                    nc.sync.dma_start(
                        qr_sb[32 * t:32 * t + KQ, :], qrep.ap()[b, t]
                    )
                if "gather" in FEATURES:
                    for cc in range(8):
                        nc.gpsimd.indirect_dma_start(
                            out=cand[0:112, D * cc:D * (cc + 1)],
                            out_offset=None,
                            in_=kfs[b].ap(),
                            in_offset=bass.IndirectOffsetOnAxis(
                                ap=r32[0:112, cc:cc + 1], axis=0
                            ),
                        )
                prod = outp.tile([128, 8 * D], F32, tag="prod")
                nc.vector.tensor_tensor(
                    prod[0:112, :], cand[0:112, :], qr_sb[0:112, :],
                    op=mybir.AluOpType.mult,
                )
                dots = small.tile([128, 8], F32, tag="dots")
                nc.vector.tensor_reduce(
                    dots[0:112, :],
                    prod[0:112, :].rearrange("p (c d) -> p c d", d=D),
                    axis=mybir.AxisListType.X, op=mybir.AluOpType.add,
                )
                m = small.tile([128, 1], F32, tag="m")
                nc.vector.tensor_reduce(
                    m[0:112], dots[0:112, :],
                    axis=mybir.AxisListType.X, op=mybir.AluOpType.max,
                )
                mask = small.tile([128, 8], U32, tag="mask")
                nc.vector.tensor_tensor(
                    mask[0:112, :], dots[0:112, :],
                    m[0:112].to_broadcast([112, 8]),
                    op=mybir.AluOpType.is_ge,
                )
                idxf = small.tile([128, 8], F32, tag="idxf")
                nc.vector.tensor_copy(idxf[0:112, :], idx8[0:112, :])
                sel = small.tile([128, 8], F32, tag="sel")
                nc.vector.memset(sel[:], 16384.0)
                nc.vector.copy_predicated(
                    sel[0:112, :], mask[0:112, :], idxf[0:112, :]
                )
                if DEBUG and b == 0:
                    nc.sync.dma_start(dbg_idx8.ap()[0:112], idx8[0:112, :])
                    nc.sync.dma_start(dbg_r32.ap()[0:112], r32[0:112, :])
                    nc.sync.dma_start(dbg_cand.ap()[0:112], cand[0:112, :])
                    nc.sync.dma_start(dbg_qr.ap()[0:112], qr_sb[0:112, :])
                    nc.sync.dma_start(dbg_dots.ap()[0:112], dots[0:112, :])
                    nc.sync.dma_start(dbg_m.ap()[0:112], m[0:112, :])
                    nc.sync.dma_start(dbg_mask.ap()[0:112], mask[0:112, :])
                    nc.sync.dma_start(dbg_sel.ap()[0:112], sel[0:112, :])
                tokf = small.tile([128, 1], F32, tag="tokf")
                nc.vector.tensor_reduce(
                    tokf[0:112], sel[0:112, :],
                    axis=mybir.AxisListType.X, op=mybir.AluOpType.min,
                )
                toku = small.tile([128, 1], U32, tag="toku")
                nc.vector.tensor_copy(toku[0:112], tokf[0:112])
                for t in range(T):
                    nc.gpsimd.dma_start(
                        tok_o.ap()[b, t], toku[32 * t:32 * t + KQ, :]
                    )

    nc.compile()
    return nc


def _prep_in_maps(q, k, v):
    ident = np.eye(128, dtype=ml_dtypes.bfloat16)
    in_maps = []
    for c in range(NCORES):
        sl = slice(BPC * c, BPC * (c + 1))
        qc, kc, vc = q[sl], k[sl], v[sl]
        # ktb_pre[b, t, 64h+d, x] = k[b, 4096h+x, t, d]
        ktb = (
            kc.transpose(0, 2, 3, 1)                      # (b, t, d, v)
            .reshape(BPC, T, D, 2, V // 2)
            .transpose(0, 1, 3, 2, 4)                     # (b, t, h, d, x)
            .reshape(BPC, T, 128, V // 2)
        )
        ktb = np.ascontiguousarray(ktb).astype(ml_dtypes.bfloat16)
        # vb_pre[b, vc, p, s*256+x] = v[b, 2048*vc + 128*s + p, t, d]
        vbb = (
            vc.reshape(BPC, 4, 16, 128, T * D)            # (b, vc, s, p, x)
            .transpose(0, 1, 3, 2, 4)                     # (b, vc, p, s, x)
            .reshape(BPC, 4, 128, 4096)
        )
        vbb = np.ascontiguousarray(vbb).astype(ml_dtypes.bfloat16)
        # qtb[b, 64h+d, 16t+j] = q[b, j, t, d], duplicated across h
        qq = np.ascontiguousarray(qc.transpose(0, 3, 2, 1)).reshape(BPC, D, T * KQ)
        qtb = np.concatenate([qq, qq], axis=1).astype(ml_dtypes.bfloat16)
        # qrep[b, t, j, 8*D] = q[b, j, t, :] tiled 8x
        qr = np.ascontiguousarray(qc.transpose(0, 2, 1, 3))          # (b, t, j, d)
        qrep = np.broadcast_to(
            qr[:, :, :, None, :], (BPC, T, KQ, 8, D)
        ).reshape(BPC, T, KQ, 8 * D).copy()
        im = dict(
            ktb=ktb, vb=vbb, qtb=qtb, qrep=qrep,
            identb=ident,
        )
        for b in range(BPC):
            im[f"kf{b}"] = np.ascontiguousarray(kc[b].reshape(V * T, D))
        in_maps.append(im)
    return in_maps


def _postprocess(res):
    out = np.empty((BS, KQ, T, D), dtype=np.float32)
    tokens = np.empty((BS, KQ, T), dtype=np.int32)
    for c in range(NCORES):
        r = res.results[c]
        oo = r["out_o"]            # (BPC, T, KQ, D)
        tt = r["tok_o"][..., 0]    # (BPC, T, KQ)
        for b in range(BPC):
            out[BPC * c + b] = oo[b].transpose(1, 0, 2)
            tokens[BPC * c + b] = tt[b].astype(np.int64).T.astype(np.int32)
    return out, tokens


def kernel(q: np.ndarray, k: np.ndarray, v: np.ndarray):
    q = np.asarray(q, dtype=np.float32)
    k = np.asarray(k, dtype=np.float32)
    v = np.asarray(v, dtype=np.float32)
    if "nc" not in _CACHED:
        _CACHED["nc"] = _build()
    in_maps = _prep_in_maps(q, k, v)
    res = run_bass_kernel_spmd(_CACHED["nc"], in_maps, core_ids=list(range(NCORES)))
    return _postprocess(res)


def run_traced(q: np.ndarray, k: np.ndarray, v: np.ndarray):
    """Like kernel() but with NTFF tracing; returns BassKernelResults."""
    q = np.asarray(q, dtype=np.float32)
    k = np.asarray(k, dtype=np.float32)
    v = np.asarray(v, dtype=np.float32)
    if "nc" not in _CACHED:
        _CACHED["nc"] = _build()
    in_maps = _prep_in_maps(q, k, v)
    return run_bass_kernel_spmd(
        _CACHED["nc"], in_maps, core_ids=list(range(NCORES)), trace=True
    )
